# revision 4
# baseline (speedup 1.0000x reference)
"""Conv4d (B=2, Ci=32, Co=64, 16^4 spatial, k=3^4, stride 1, pad 1) on 8
Trainium2 NeuronCores.

Sharding: 8 cores = batch(2) x T-quarters(4). Each core computes
out[64co, 4t, 16d, 16h, 16w] for its (b, t-quarter).

The 81 taps are covered by three passes:
  A: (kt,kd) in {(0,0),(0,1),(0,2),(1,0)} packed into K=128 (partition
     group g holds x shifted by combo g), M=64, one matmul per (kh,kw).
  B: (kt,kd) in {(1,2),(2,0),(2,1),(2,2)} likewise on a second layout.
  C: (kt,kd)=(1,1) as K=32 matmuls on a cropped quadrant layout
     (partition group r = D-quarter), issued as 9 waves of 4 row-tiled
     matmuls sharing ONE [128,64] LDWEIGHTS per wave.
Shared-LDWEIGHTS surgery: the tile legalizer splits every matmul into
LDWEIGHTS+MATMUL; after scheduling we delete the redundant LDWEIGHTS of
same-weight matmul runs (keeping the first of each group / the explicit
per-wave load), which removes the per-tile weight-reload serialization
that made pass C waves cost ~380ns instead of ~215ns.
Other changes vs the 111us baseline: PE warmup matmuls on a memset
scratch tile (no DMA dependency) so the HAM clock gate opens during the
DMA ramp instead of 14us into the matmul stream; bf16 output staging
(halves output DMA + speeds the DVE epilogue; upcast on host); final
batch split into two 2-unit batches so the tail epilogues stagger.
"""
import sys

sys.path.insert(0, "/opt/trn_rl_repo")
import numpy as np
import ml_dtypes

N_CORES = 8
KHW = [(kh, kw) for kh in range(3) for kw in range(3)]
A_COMBOS = [(0, 0), (0, 1), (0, 2), (1, 0)]
B_COMBOS = [(1, 2), (2, 0), (2, 1), (2, 2)]

_NC = None


def _ldw_surgery(nc, mm_req, expl_eff, wave_groups, mybir):
    """Post-scheduling LDWEIGHTS surgery.

    1. Move each explicit (per-C-wave) LDW to just before the earliest-
       scheduled matmul of its wave, leaving its waits at the old slot.
    2. Walk the PE stream simulating the 4x2 subarray weight state
       (32-row x 64-col subtiles) and delete every LDWEIGHTS whose load
       is a no-op. Deleting a no-op load cannot change semantics, so
       this is safe under any scheduler order.

    mm_req: mm name -> list of ((r, c), key) weight requirements.
    expl_eff: explicit ldw name -> list of ((r, c), key) effects.
    wave_groups: explicit ldw name -> list of its wave's mm names.
    """
    # pre-pass: map each auto LDW to its matmul's requirement while the
    # legalizer's adjacency still holds (before any repositioning).
    auto_eff = {}
    for blk in nc.main_func.blocks:
        pe = [i for i in blk.instructions
              if type(i).__name__ in ("InstLdweights", "InstMatmult")]
        for k, inst in enumerate(pe):
            if type(inst).__name__ != "InstLdweights":
                continue
            if inst.name in expl_eff:
                continue
            assert k + 1 < len(pe) and \
                type(pe[k + 1]).__name__ == "InstMatmult", \
                "auto ldw not adjacent to a matmul"
            auto_eff[inst.name] = mm_req[pe[k + 1].name]

    for blk in nc.main_func.blocks:
        names = {i.name for i in blk.instructions}
        for ldw_name, mms in wave_groups.items():
            if ldw_name not in names:
                continue
            insts = list(blk.instructions)
            pos = {inst.name: k for k, inst in enumerate(insts)}
            tgt_idx = min((pos[m] for m in mms if m in pos), default=None)
            if tgt_idx is None:
                continue
            ldw = insts[pos[ldw_name]]
            old_idx = pos[ldw_name]
            if tgt_idx <= old_idx:
                continue
            si = ldw.sync_info
            waits = list(si.on_wait) if si is not None else []
            if waits:
                tgt = None
                for j in range(old_idx + 1, len(insts)):
                    if insts[j].engine == ldw.engine and j != tgt_idx:
                        tgt = insts[j]
                        break
                if tgt is None:
                    tgt = insts[tgt_idx]
                tsi = tgt.sync_info
                tw = list(tsi.on_wait) if tsi is not None else []
                tu = list(tsi.on_update) if tsi is not None else []
                tgt.sync_info = mybir.SyncInfo(on_wait=waits + tw,
                                               on_update=tu)
                ups = list(si.on_update)
                ldw.sync_info = mybir.SyncInfo(on_wait=[], on_update=ups)
            mm_inst = insts[tgt_idx]
            blk.instructions.remove(ldw)
            idx = blk.instructions.index(mm_inst)
            blk.instructions.insert(idx, ldw)

    n_del = 0
    for blk in nc.main_func.blocks:
        state = {}
        insts = list(blk.instructions)
        pe = [(k, i) for k, i in enumerate(insts)
              if type(i).__name__ in ("InstLdweights", "InstMatmult")]
        doomed = []
        for pk, (k, inst) in enumerate(pe):
            tn = type(inst).__name__
            if tn == "InstMatmult":
                continue
            eff = expl_eff.get(inst.name) or auto_eff[inst.name]
            if all(state.get(sub) == key for sub, key in eff):
                doomed.append(inst)
                n_del += 1
            else:
                for sub, key in eff:
                    state[sub] = key
        for inst in doomed:
            si = inst.sync_info
            waits = list(si.on_wait) if si is not None else []
            ups = list(si.on_update) if si is not None else []
            if waits or ups:
                idx = blk.instructions.index(inst)
                tgt = None
                for j in range(idx + 1, len(blk.instructions)):
                    if blk.instructions[j].engine == inst.engine:
                        tgt = blk.instructions[j]
                        break
                assert tgt is not None
                tsi = tgt.sync_info
                tw = list(tsi.on_wait) if tsi is not None else []
                tu = list(tsi.on_update) if tsi is not None else []
                tgt.sync_info = mybir.SyncInfo(on_wait=waits + tw,
                                               on_update=ups + tu)
            blk.instructions.remove(inst)
    return n_del


def _build():
    global _NC
    if _NC is not None:
        return _NC
    import concourse.bacc as bacc
    import concourse.tile as tile
    from concourse import mybir

    f32 = mybir.dt.float32
    bf16 = mybir.dt.bfloat16

    nc = bacc.Bacc("TRN2", debug=False, target_bir_lowering=False,
                   num_devices=N_CORES)
    xqa = nc.dram_tensor("xqa", [128, 20736], bf16, kind="ExternalInput")
    xqb = nc.dram_tensor("xqb", [128, 20736], bf16, kind="ExternalInput")
    xqc = nc.dram_tensor("xqc", [128, 5184], bf16, kind="ExternalInput")
    wa = nc.dram_tensor("wa", [128, 576], bf16, kind="ExternalInput")
    wb = nc.dram_tensor("wb", [128, 576], bf16, kind="ExternalInput")
    wc = nc.dram_tensor("wc", [128, 576], bf16, kind="ExternalInput")
    bq = nc.dram_tensor("biasq", [128, 1], f32, kind="ExternalInput")
    out = nc.dram_tensor("out", [64, 16384], bf16, kind="ExternalOutput")

    mm_req = {}          # mm name -> [((r, c), key), ...]
    expl_eff = {}        # explicit ldw name -> [((r, c), key), ...]
    wave_groups = {}     # explicit ldw name -> [its wave's mm names]

    with tile.TileContext(nc) as tc:
        with tc.tile_pool(name="xp", bufs=1) as xp, \
             tc.tile_pool(name="wp", bufs=1) as wp, \
             tc.tile_pool(name="op", bufs=8) as op_, \
             tc.tile_pool(name="pp", bufs=8, space="PSUM") as pp:
            wat = wp.tile([128, 576], bf16)
            wbt = wp.tile([128, 576], bf16)
            wct = wp.tile([128, 576], bf16)
            btile = wp.tile([128, 1], f32)
            scr = wp.tile([128, 512], bf16)
            xat = xp.tile([128, 20736], bf16)
            xbt = xp.tile([128, 20736], bf16)
            xct = xp.tile([128, 5184], bf16)

            # PE warmup with no DMA dependency: memset scratch, then a
            # run of matmuls to open the HAM clock gate (~3.4us busy)
            # while the input DMA streams in.
            nc.vector.memset(scr[:], 0.0)
            wu = pp.tile([64, 512], f32, tag="ps", name="warmup")
            NWU = 8
            for i in range(NWU):
                b = nc.tensor.matmul(wu[:], scr[:, 0:64], scr[:],
                                     start=i == 0, stop=i == NWU - 1,
                                     tile_position=(0, 0))
                mm_req[b.ins.name] = [((r, 0), ("scr", 0, r))
                                      for r in range(4)]

            # Issue order == arrival order (one FIFO input queue feeding
            # all 16 DMA engines). First matmuls need wa + A[t0=0,d 0..3]
            # only, so those go first, d-chunked, on the sync queue.
            nc.sync.dma_start(wat[:], wa.ap()[:])
            nc.gpsimd.dma_start(wbt[:], wb.ap()[:])
            for q in range(4):
                nc.sync.dma_start(xat[:, q * 1296:(q + 1) * 1296],
                                  xqa.ap()[:, q * 1296:(q + 1) * 1296])
                nc.gpsimd.dma_start(
                    xbt[:, q * 1296:(q + 1) * 1296],
                    xqb.ap()[:, q * 1296:(q + 1) * 1296])
            nc.gpsimd.dma_start(wct[:], wc.ap()[:])
            nc.gpsimd.dma_start(btile[:], bq.ap()[:])
            nc.gpsimd.dma_start(xct[:, 0:1296], xqc.ap()[:, 0:1296])
            for t0 in range(1, 4):
                nc.sync.dma_start(xat[:, t0 * 5184:(t0 + 1) * 5184],
                                  xqa.ap()[:, t0 * 5184:(t0 + 1) * 5184])
                nc.gpsimd.dma_start(xbt[:, t0 * 5184:(t0 + 1) * 5184],
                                    xqb.ap()[:, t0 * 5184:(t0 + 1) * 5184])
                nc.gpsimd.dma_start(
                    xct[:, t0 * 1296:(t0 + 1) * 1296],
                    xqc.ap()[:, t0 * 1296:(t0 + 1) * 1296])

            xav = xat.rearrange("p (t d h w) -> p t d h w",
                                t=4, d=16, h=18, w=18)
            xbv = xbt.rearrange("p (t d h w) -> p t d h w",
                                t=4, d=16, h=18, w=18)
            xcv = xct.rearrange("p (t d h w) -> p t d h w",
                                t=4, d=4, h=18, w=18)

            # (even tap, odd tap) pairs; (A j8, B j0) bridges the passes.
            PAIRS = [((0, 0), (0, 1)), ((0, 2), (0, 3)),
                     ((0, 4), (0, 5)), ((0, 6), (0, 7)),
                     ((0, 8), (1, 0)), ((1, 1), (1, 2)),
                     ((1, 3), (1, 4)), ((1, 5), (1, 6)),
                     ((1, 7), (1, 8))]

            # batches: 7 quads, then the last quad split into two duos so
            # the tail epilogues stagger.
            batches = [(bi // 2, [bi % 2 + 2 * k for k in range(4)])
                       for bi in range(7)]
            batches += [(3, [1, 3]), (3, [5, 7])]

            for bnum, (to, units) in enumerate(batches):
                ps = {dp: pp.tile([128, 512], f32, tag="ps",
                                  name=f"ps_{bnum}_{dp}") for dp in units}
                nch = {dp: [0, 0] for dp in units}
                tot = {dp: [14, 13] for dp in units}

                def mm_ab(pi, j, dp):
                    wt, xv = ((wat, xav), (wbt, xbv))[pi]
                    kh, kw = KHW[j]
                    c = (j + pi) % 2
                    nch[dp][c] += 1
                    b = nc.tensor.matmul(
                        ps[dp][64 * c:64 * c + 64, :],
                        wt[:, j * 64:(j + 1) * 64],
                        xv[:, to, 2 * dp:2 * dp + 2,
                           kh:kh + 16, kw:kw + 16],
                        start=nch[dp][c] == 1,
                        stop=nch[dp][c] == tot[dp][c],
                        tile_position=(0, 64 * c))
                    mm_req[b.ins.name] = [((r, c), ("ab"[pi], j, r))
                                          for r in range(4)]

                # A/B: per pair, the 4 units' even-half MMs share one
                # LDWEIGHTS (the later ones dedupe to no-ops), same odd.
                for (pa, ja), (pb, jb) in PAIRS:
                    for i, dp in enumerate(units):
                        mm_ab(pa, ja, dp)
                        mm_ab(pb, jb, dp)

                # C: 9 waves; each wave = one explicit [128,64] LDW at
                # (0, 64c) + one K=32 row-tiled MM per unit (row group
                # r=dp//2), auto LDWs deleted.
                for j, (kh, kw) in enumerate(KHW):
                    c = j % 2
                    ld = nc.tensor.ldweights(
                        wct[:, j * 64:(j + 1) * 64],
                        tile_position=(0, 64 * c))
                    expl_eff[ld.ins.name] = [((r, c), ("c", j, r))
                                             for r in range(4)]
                    wave_groups[ld.ins.name] = []
                    for i, dp in enumerate(units):
                        r = dp // 2
                        ldd = 2 * (dp % 2)
                        nch[dp][c] += 1
                        b = nc.tensor.matmul(
                            ps[dp][64 * c:64 * c + 64, :],
                            wct[32 * r:32 * r + 32, j * 64:(j + 1) * 64],
                            xcv[32 * r:32 * r + 32, to, ldd:ldd + 2,
                                kh:kh + 16, kw:kw + 16],
                            start=nch[dp][c] == 1,
                            stop=nch[dp][c] == tot[dp][c],
                            tile_position=(32 * r, 64 * c))
                        mm_req[b.ins.name] = [((r, c), ("c", j, r))]
                        wave_groups[ld.ins.name].append(b.ins.name)

                # epilogue: ACT adds bias to the odd half, DVE adds the
                # halves (bf16 out), DMA out.
                for k, dp in enumerate(units):
                    ob = op_.tile([64, 512], f32, tag="ob",
                                  name=f"ob_{bnum}_{dp}")
                    oa = op_.tile([64, 512], bf16, tag="oa",
                                  name=f"oa_{bnum}_{dp}")
                    nc.scalar.activation(
                        ob[:], ps[dp][64:128, :],
                        mybir.ActivationFunctionType.Identity,
                        bias=btile[64:128, 0:1])
                    nc.vector.tensor_tensor(oa[:], ps[dp][0:64, :],
                                            ob[:], mybir.AluOpType.add)
                    off = to * 4096 + dp * 512
                    dq = (nc.sync, nc.gpsimd, nc.scalar,
                          nc.sync)[dp % 4] if dp >= 4 else \
                        (nc.gpsimd, nc.scalar, nc.sync, nc.gpsimd)[dp % 4]
                    dq.dma_start(out.ap()[:, off:off + 512], oa[:])

    # ---- post-scheduling surgery: place explicit per-wave LDWs, then
    # delete every LDWEIGHTS that is a no-op on the simulated PE weight
    # state (safe under any scheduler order).
    _ldw_surgery(nc, mm_req, expl_eff, wave_groups, mybir)
    nc.compile()
    _NC = nc
    return nc


def _prep_inputs(x, weight, bias):
    x = np.asarray(x, dtype=np.float32)
    weight = np.asarray(weight, dtype=np.float32)
    bias = np.asarray(bias, dtype=np.float32)

    def wpack(kt, kd):
        # [32ci, 9khw * 64co]
        return np.ascontiguousarray(
            weight[:, :, kt, kd].reshape(64, 32, 9).transpose(1, 2, 0)
        ).reshape(32, 576)

    wa = np.concatenate([wpack(kt, kd) for kt, kd in A_COMBOS], axis=0)
    wb = np.concatenate([wpack(kt, kd) for kt, kd in B_COMBOS], axis=0)
    wc = np.concatenate([wpack(1, 1)] * 4, axis=0)
    wa = wa.astype(ml_dtypes.bfloat16)
    wb = wb.astype(ml_dtypes.bfloat16)
    wc = wc.astype(ml_dtypes.bfloat16)
    bq = np.concatenate([np.zeros((64, 1), np.float32),
                     bias.reshape(64, 1)]).astype(np.float32)

    in_maps = []
    for b in range(2):
        xpad = np.pad(x[b], ((0, 0), (1, 1), (1, 1), (1, 1), (1, 1)))
        for tq in range(4):
            xt = xpad[:, 4 * tq:4 * tq + 6]  # [32, 6t, 18d, 18, 18]
            xa = np.empty((128, 20736), ml_dtypes.bfloat16)
            xb = np.empty((128, 20736), ml_dtypes.bfloat16)
            for g, (kt, kd) in enumerate(A_COMBOS):
                xa[32 * g:32 * g + 32] = \
                    xt[:, kt:kt + 4, kd:kd + 16].reshape(32, -1)
            for g, (kt, kd) in enumerate(B_COMBOS):
                xb[32 * g:32 * g + 32] = \
                    xt[:, kt:kt + 4, kd:kd + 16].reshape(32, -1)
            # cropped quadrant layout for pass C (kt=kd=1):
            # t planes 1..4, per-quarter padded-d planes 4r+1..4r+4
            xc = np.empty((128, 5184), ml_dtypes.bfloat16)
            for r in range(4):
                xc[32 * r:32 * r + 32] = \
                    xt[:, 1:5, 4 * r + 1:4 * r + 5].reshape(32, -1)
            in_maps.append({"xqa": xa, "xqb": xb, "xqc": xc,
                            "wa": wa, "wb": wb, "wc": wc, "biasq": bq})
    return in_maps


def run_spmd(x, weight, bias, trace=False, trace_cores=None, tmpdir=None):
    """Returns (output ndarray, BassKernelResults)."""
    from concourse.bass_utils import run_bass_kernel_spmd
    nc = _build()
    in_maps = _prep_inputs(x, weight, bias)
    res = run_bass_kernel_spmd(nc, in_maps, core_ids=list(range(N_CORES)),
                               trace=trace, trace_cores=trace_cores,
                               tmpdir=tmpdir)
    out = np.empty((2, 64, 16, 16, 16, 16), np.float32)
    for c in range(N_CORES):
        b, tq = c // 4, c % 4
        out[b, :, 4 * tq:4 * tq + 4] = \
            res.results[c]["out"].astype(np.float32).reshape(64, 4, 16, 16, 16)
    return out, res


def kernel(x, weight, bias):
    out, _ = run_spmd(x, weight, bias)
    return out
